# revision 2
# baseline (speedup 1.0000x reference)
"""Trainium2 Bass kernel for nn_PitchRegisterTracker.

Algorithm notes
---------------
The reference maintains a size-1000 circular buffer of log2-pitches of the
valid (>0) frames, then normalizes every valid frame by the buffer's
mean/unbiased-std.  Because slot j keeps the *highest-rank* writer, the full
buffer is exactly the last min(1000, n_valid) valid elements.  So:

  phase A: stats (mean/std of ln-pitch) over the last 1000 valid elements.
           Only a small tail window of the input can contain them; every core
           redundantly computes the same stats from the same tail (no
           collectives needed).
  phase B: fully data-parallel elementwise map
           out = exp(sc * ln(p) + bi),  out(0) = 0 via exp(ln(0)) = exp(-inf).

where, with ln-space stats meanL/stdL over the selected values:
  sc = TLS*ln2/stdL   bi = ln2*TLM - meanL*sc
matching the reference's exp2((log2p - mean2)/std2 * TLS + TLM).

Performance notes
-----------------
The kernel is HBM-bandwidth bound (16 MiB in + 16 MiB out per core).  DMA
packets stripe over the 16 SDMA engines by SBUF partition (engine k serves a
fixed set of 8 partitions).  Profiles show SDMA engine 15 runs ~20% slower
than the others (~745ns vs 610ns per 16 KiB packet), so a uniform layout
makes everything wait on it.  We therefore give engine 15's partitions
({92..95, 124..127}) proportionally shorter rows: the per-core shard is laid
out as a fast block (120 partitions x CF) and a slow block (8 partitions x
CS), with CS/CF ~ the measured speed ratio.  Streaming loads+stores all ride
SP's HWDGE ring (loads queued first, stores strictly after - the FIFO keeps
the bus work-conserving), leaving ACT free for the Ln/Exp compute.  Phase-A
inputs ride ACT's ring, which is otherwise idle at startup.
"""

import sys

for _p in ("/opt/trn_rl_repo", "/root/.axon_site/_ro/trn_rl_repo"):
    if _p not in sys.path:
        sys.path.insert(0, _p)

import numpy as np

import concourse.bass as bass
import concourse.mybir as mybir
from concourse import tile
from concourse.bass_utils import run_bass_kernel_spmd

AF = mybir.ActivationFunctionType
OP = mybir.AluOpType
F32 = mybir.dt.float32

N_CORES = 8
BUF = 1000
LN2 = 0.693147  # the reference's constant, used only inside TLS
TARGET_LOG_MEAN = float(np.log2(200.0))
TARGET_LOG_STD = 40.0 / (200.0 * LN2)
LN2_T = float(np.log(2.0))  # true ln 2

# Banded streaming layout (see "Performance notes").
NT = 8  # tiles per core
FF = 4144  # fast-band columns per tile (120 partitions)
FS = 3376  # slow-band columns per tile (8 partitions: engine 15's)
CF = NT * FF  # fast-band row length
CS = NT * FS  # slow-band row length
NFAST = 120 * CF  # elements in the fast block
SHARD = NFAST + 8 * CS
assert 120 * FF + 8 * FS == SHARD // NT


def _legalize_sync_waits(nc, maxw=1):
    """This container's walrus accepts at most one sync-wait command per
    instruction; split extra waits into preceding same-engine NOPs."""
    n = 0
    for f in nc.m.functions:
        for bb in f.blocks:
            insts = bb.instructions
            newlist = []
            for inst in insts:
                si = inst.sync_info
                if si is not None and si.on_wait and len(si.on_wait) > maxw:
                    waits = list(si.on_wait)
                    rest = waits[-maxw:]
                    head = waits[:-maxw]
                    k = 0
                    while head:
                        chunk, head = head[:maxw], head[maxw:]
                        nop = mybir.InstNoOp(
                            name=f"{inst.name}-ws{k}",
                            sync_info=mybir.SyncInfo(
                                on_wait=list(chunk), on_update=[]
                            ),
                            engine=inst.engine,
                            bass_nofuse=True,
                        )
                        nc.register_instruction(nop)
                        newlist.append(nop)
                        k += 1
                        n += 1
                    si.on_wait.clear()
                    si.on_wait.extend(rest)
                newlist.append(inst)
            insts[:] = newlist
    return n


def _build_program(tail):
    """One SPMD program: per-core banded shard -> banded out, with the
    global tail [tail] + constants replicated to every core."""
    tailc = tail // 128

    nc = bass.Bass()
    xf = nc.dram_tensor("xf", [NFAST], F32, kind="ExternalInput")
    xg = nc.dram_tensor("xg", [8 * CS], F32, kind="ExternalInput")
    xt = nc.dram_tensor("xt", [tail], F32, kind="ExternalInput")
    lts = nc.dram_tensor("lts", [128, 256], F32, kind="ExternalInput")
    yf = nc.dram_tensor("yf", [NFAST], F32, kind="ExternalOutput")
    yg = nc.dram_tensor("yg", [8 * CS], F32, kind="ExternalOutput")

    xft = xf.rearrange("(p c) -> p c", p=120)
    xgt = xg.rearrange("(p c) -> p c", p=8)
    yft = yf.rearrange("(p c) -> p c", p=120)
    ygt = yg.rearrange("(p c) -> p c", p=8)
    xtt = xt.rearrange("(p c) -> p c", p=128)

    sc_clamp = TARGET_LOG_STD * 1e7
    exp_bias = float(np.log(TARGET_LOG_STD * LN2_T))
    ln2sq = LN2_T * LN2_T

    with tile.TileContext(nc) as tc:
        with (
            tc.tile_pool(name="const", bufs=1) as cpool,
            tc.tile_pool(name="stat", bufs=1) as spool,
            tc.tile_pool(name="psum", bufs=1, space="PSUM") as ppool,
            tc.tile_pool(name="big", bufs=NT) as bpool,
        ):
            # ---------------- phase A: stats over last BUF valid in tail
            # phase-A inputs ride ACT's HWDGE ring (idle at startup) so the
            # first big phase-B load is the very first trigger on SP's ring
            ltst = cpool.tile([128, 256], F32)
            tailt = cpool.tile([128, tailc], F32)
            nc.scalar.dma_start(ltst[:], lts[:])
            nc.scalar.dma_start(tailt[:], xtt[:])

            zer = cpool.tile([128, tailc], F32)
            nc.vector.memset(zer[:], 0.0)

            mask = cpool.tile([128, tailc], F32)
            nc.vector.tensor_scalar(mask[:], tailt[:], 0.0, None, OP.is_gt)
            t1 = cpool.tile([128, tailc], F32)
            nc.vector.tensor_scalar(t1[:], tailt[:], 1.0, None, OP.max)
            lnp = cpool.tile([128, tailc], F32)
            nc.scalar.activation(lnp[:], t1[:], AF.Ln)

            # per-partition inclusive prefix count of valid
            pre = cpool.tile([128, tailc], F32)
            nc.vector.tensor_tensor_scan(
                pre[:], mask[:], zer[:], 0.0, OP.add, OP.add
            )

            # cross-partition exclusive prefix + total, via PE matmuls
            ps_rexc = ppool.tile([128, 1], F32)
            ps_vb = ppool.tile([128, 1], F32)
            last = pre[:, tailc - 1 : tailc]
            nc.tensor.matmul(ps_rexc[:], ltst[:, 0:128], last)
            nc.tensor.matmul(ps_vb[:], ltst[:, 128:256], last)
            rexc = spool.tile([128, 1], F32)
            vb = spool.tile([128, 1], F32)
            nc.vector.tensor_copy(rexc[:], ps_rexc[:])
            nc.vector.tensor_copy(vb[:], ps_vb[:])

            # w = V - rexc - BUF ; select valid lanes with global prefix > w
            w = spool.tile([128, 1], F32)
            nc.vector.tensor_scalar(
                w[:], vb[:], rexc[:, 0:1], float(BUF), OP.subtract, OP.subtract
            )
            selg = cpool.tile([128, tailc], F32)
            nc.vector.tensor_scalar(selg[:], pre[:], w[:, 0:1], None, OP.is_gt)
            sel = cpool.tile([128, tailc], F32)
            nc.vector.tensor_tensor(sel[:], selg[:], mask[:], OP.mult)

            # one-pass moments: cnt, sum(t), sum(t^2) over selected lanes,
            # rows packed into one [128,3] tile -> single broadcast matmul
            stats = spool.tile([128, 3], F32)
            slog = cpool.tile([128, tailc], F32)
            slog2 = cpool.tile([128, tailc], F32)
            nc.vector.tensor_reduce(
                stats[:, 0:1], sel[:], mybir.AxisListType.X, OP.add
            )
            nc.vector.tensor_tensor(slog[:], sel[:], lnp[:], OP.mult)
            nc.vector.tensor_reduce(
                stats[:, 1:2], slog[:], mybir.AxisListType.X, OP.add
            )
            nc.vector.tensor_tensor(slog2[:], slog[:], lnp[:], OP.mult)
            nc.vector.tensor_reduce(
                stats[:, 2:3], slog2[:], mybir.AxisListType.X, OP.add
            )
            ps_st = ppool.tile([128, 3], F32)
            nc.tensor.matmul(ps_st[:], ltst[:, 128:256], stats[:, 0:3])
            bst = spool.tile([128, 3], F32)
            nc.vector.tensor_copy(bst[:], ps_st[:])
            cntb = bst[:, 0:1]
            s1b = bst[:, 1:2]
            s2b = bst[:, 2:3]

            # 1/x via exp(-ln x) on ACT: this walrus rejects the custom-DVE
            # reciprocal encoding, and x (a count >= 1) is exact enough here
            cfl = spool.tile([128, 1], F32)
            nc.vector.tensor_scalar(cfl[:], cntb, 1.0, None, OP.max)
            lncf = spool.tile([128, 1], F32)
            nc.scalar.activation(lncf[:], cfl[:], AF.Ln)
            rcp1 = spool.tile([128, 1], F32)
            nc.scalar.activation(rcp1[:], lncf[:], AF.Exp, scale=-1.0)
            meanl = spool.tile([128, 1], F32)
            nc.vector.tensor_tensor(meanl[:], s1b, rcp1[:], OP.mult)

            # unbiased variance, one-pass: (s2 - s1*mean) / max(cnt-1, 1)
            smean = spool.tile([128, 1], F32)
            nc.vector.tensor_tensor(smean[:], s1b, meanl[:], OP.mult)
            diff = spool.tile([128, 1], F32)
            nc.vector.tensor_tensor(diff[:], s2b, smean[:], OP.subtract)
            diffc = spool.tile([128, 1], F32)
            nc.vector.tensor_scalar(diffc[:], diff[:], 0.0, None, OP.max)

            den = spool.tile([128, 1], F32)
            nc.vector.tensor_scalar(
                den[:], cntb, 1.0, 1.0, OP.subtract, OP.max
            )
            lnden = spool.tile([128, 1], F32)
            nc.scalar.activation(lnden[:], den[:], AF.Ln)
            rcp2 = spool.tile([128, 1], F32)
            nc.scalar.activation(rcp2[:], lnden[:], AF.Exp, scale=-1.0)
            varl = spool.tile([128, 1], F32)
            nc.vector.tensor_tensor(varl[:], diffc[:], rcp2[:], OP.mult)

            # count<=1 -> std2 := 1  (stdL := ln2), via varL += ind*ln2^2
            ind = spool.tile([128, 1], F32)
            nc.vector.tensor_scalar(
                ind[:], cntb, 1.5, ln2sq, OP.is_lt, OP.mult
            )
            varp = spool.tile([128, 1], F32)
            nc.vector.tensor_tensor(varp[:], varl[:], ind[:], OP.add)

            # sc = TLS*ln2/sqrt(varp) = exp(-0.5*ln(varp) + ln(TLS*ln2))
            lnv = spool.tile([128, 1], F32)
            nc.scalar.activation(lnv[:], varp[:], AF.Ln)
            ebias = spool.tile([128, 1], F32)
            nc.vector.memset(ebias[:], exp_bias)
            sc_r = spool.tile([128, 1], F32)
            nc.scalar.activation(
                sc_r[:], lnv[:], AF.Exp, scale=-0.5, bias=ebias[:, 0:1]
            )
            sc = spool.tile([128, 1], F32)
            nc.vector.tensor_scalar(sc[:], sc_r[:], sc_clamp, None, OP.min)
            mb = spool.tile([128, 1], F32)
            nc.vector.tensor_tensor(mb[:], meanl[:], sc[:], OP.mult)
            bi = spool.tile([128, 1], F32)
            nc.vector.tensor_scalar(
                bi[:], mb[:], -1.0, LN2_T * TARGET_LOG_MEAN, OP.mult, OP.add
            )

            # ---------------- phase B: streamed elementwise map
            # All loads queue on SP's ring first; stores follow strictly
            # after.  Slow-band columns [FS:FF] hold stale data - Ln/Exp
            # runs over them harmlessly and stores skip them.
            tiles = []
            for i in range(NT):
                tl = bpool.tile([128, FF], F32, tag="work")
                f0, f1 = i * FF, (i + 1) * FF
                g0, g1 = i * FS, (i + 1) * FS
                nc.sync.dma_start(tl[0:92, :], xft[0:92, f0:f1])
                nc.sync.dma_start(tl[96:124, :], xft[92:120, f0:f1])
                nc.sync.dma_start(tl[92:96, 0:FS], xgt[0:4, g0:g1])
                nc.sync.dma_start(tl[124:128, 0:FS], xgt[4:8, g0:g1])
                tiles.append(tl)
            for tl in tiles:
                nc.scalar.activation(tl[:, :], tl[:, :], AF.Ln)
                nc.scalar.activation(
                    tl[:, :], tl[:, :], AF.Exp, scale=sc[:, 0:1], bias=bi[:, 0:1]
                )
            for i, tl in enumerate(tiles):
                f0, f1 = i * FF, (i + 1) * FF
                g0, g1 = i * FS, (i + 1) * FS
                nc.sync.dma_start(yft[0:92, f0:f1], tl[0:92, :])
                nc.sync.dma_start(yft[92:120, f0:f1], tl[96:124, :])
                nc.sync.dma_start(ygt[0:4, g0:g1], tl[92:96, 0:FS])
                nc.sync.dma_start(ygt[4:8, g0:g1], tl[124:128, 0:FS])

    _legalize_sync_waits(nc)
    nc.finalize()
    return nc


_cache = {}


def _get_program(tail):
    if tail not in _cache:
        _cache[tail] = _build_program(tail)
    return _cache[tail]


def _consts():
    k = np.arange(128, dtype=np.float32)
    lt_strict = (k[:, None] < k[None, :]).astype(np.float32)  # [k, m]: k < m
    ones = np.ones((128, 128), np.float32)
    return np.concatenate([lt_strict, ones], axis=1)


def _prep(x):
    """Build (nc, in_maps) for the full input x."""
    n = x.shape[0]
    assert n == N_CORES * SHARD, f"unsupported size {n}"

    # tail window guaranteed to contain the last BUF valid elements
    tail = 16384
    while tail < n and int(np.count_nonzero(x[n - tail :] > 0.0)) < BUF:
        tail *= 2
    tail = min(tail, n)
    # phase-A SBUF tiles scale with the tail; beyond 2^16 elements they
    # would not fit alongside the streaming pool
    if tail > (1 << 16):
        # pathological density: synthesize an equivalent tail on the host
        # holding the last <=BUF valid values (stats are order-independent)
        vals = x[x > 0.0]
        kv = vals[-BUF:] if vals.size > BUF else vals
        tail = 16384
        fake = np.zeros(tail, np.float32)
        if kv.size:
            fake[-kv.size :] = kv
        xt = fake
    else:
        xt = x[n - tail :]

    nc = _get_program(tail)
    consts = _consts()
    in_maps = [
        {
            "xf": x[c * SHARD : c * SHARD + NFAST],
            "xg": x[c * SHARD + NFAST : (c + 1) * SHARD],
            "xt": xt,
            "lts": consts,
        }
        for c in range(N_CORES)
    ]
    return nc, in_maps


def _assemble(res):
    out = np.empty(N_CORES * SHARD, np.float32)
    for c in range(N_CORES):
        base = c * SHARD
        out[base : base + NFAST] = res.results[c]["yf"]
        out[base + NFAST : base + SHARD] = res.results[c]["yg"]
    return out


def kernel(pitch_values):
    x = np.ascontiguousarray(np.asarray(pitch_values, dtype=np.float32))
    nc, in_maps = _prep(x)
    res = run_bass_kernel_spmd(nc, in_maps, core_ids=list(range(N_CORES)))
    return _assemble(res)


# revision 7
# speedup vs baseline: 2.7759x; 2.7759x over previous
"""Trainium2 Bass kernel for nn_PitchRegisterTracker.

Algorithm notes
---------------
The reference maintains a size-1000 circular buffer of log2-pitches of the
valid (>0) frames, then normalizes every valid frame by the buffer's
mean/unbiased-std.  Because slot j keeps the *highest-rank* writer, the full
buffer is exactly the last min(1000, n_valid) valid elements.  So:

  phase A: stats (mean/std of ln-pitch) over the last 1000 valid elements.
           Only a small tail window of the input can contain them; every core
           redundantly computes the same stats from the same tail (no
           collectives needed).
  phase B: fully data-parallel elementwise map
           out = exp(sc * ln(p) + bi),  out(0) = 0 via exp(ln(0)) = exp(-inf).

where, with ln-space stats meanL/stdL over the selected values:
  sc = TLS*ln2/stdL   bi = ln2*TLM - meanL*sc
matching the reference's exp2((log2p - mean2)/std2 * TLS + TLM).

Performance notes
-----------------
The kernel is HBM-bandwidth bound (16 MiB in + 16 MiB out per core).  HWDGE
splits one DMA's descriptors over d SDMA engines, d = largest divisor of the
descriptor count <= 16, assigning contiguous chunks starting at engine 0.
Profiles show SDMA engine 15 runs ~20% slower than engines 0-14 (~745ns vs
610ns per 16 KiB packet), and with uniform [128, C] DMAs (8 descriptors per
engine) everything ends up waiting on engine 15's backlog (~13-17us).

So each tile moves in three DMAs per direction:
  A  [128, CA]                -> 128 descs, all 16 engines, 8 descs each
  B  [120, CB]                -> 120 descs, 15 engines (engine 15 idle!)
  Bb [8, CB] as [2, 8, CB/2]  ->  16 descs, all 16 engines, 1 desc each
With CB/TILE_F ~ 0.18, engine 15 carries ~0.82x the bytes of engines 0-14,
matching its measured speed, so all 16 engines finish together.

Streaming loads+stores all ride SP's HWDGE ring (all loads queued first,
stores strictly after - the FIFO keeps the bus work-conserving), leaving ACT
free for the Ln/Exp compute.  Phase-A inputs ride ACT's ring, which is
otherwise idle at startup.
"""

import sys

for _p in ("/opt/trn_rl_repo", "/root/.axon_site/_ro/trn_rl_repo"):
    if _p not in sys.path:
        sys.path.insert(0, _p)

import math

import numpy as np

import concourse.bass as bass
import concourse.mybir as mybir
from concourse import tile
from concourse.bass_utils import run_bass_kernel_spmd

AF = mybir.ActivationFunctionType
OP = mybir.AluOpType
F32 = mybir.dt.float32

N_CORES = 8
BUF = 1000
LN2 = 0.693147  # the reference's constant, used only inside TLS
TARGET_LOG_MEAN = float(np.log2(200.0))
TARGET_LOG_STD = 40.0 / (200.0 * LN2)
LN2_T = float(np.log(2.0))  # true ln 2

TILE_F = 4096
CB = 746  # columns routed via the 15-engine DMA (engine-15 skew)
CA = TILE_F - CB
PADC = 2  # SBUF gap between the two Bb chunks (prevents AP dim-merge)
TILE_W = TILE_F + PADC


def _legalize_sync_waits(nc, maxw=1):
    """This container's walrus accepts at most one sync-wait command per
    instruction; split extra waits into preceding same-engine NOPs."""
    n = 0
    for f in nc.m.functions:
        for bb in f.blocks:
            insts = bb.instructions
            newlist = []
            for inst in insts:
                si = inst.sync_info
                if si is not None and si.on_wait and len(si.on_wait) > maxw:
                    waits = list(si.on_wait)
                    rest = waits[-maxw:]
                    head = waits[:-maxw]
                    k = 0
                    while head:
                        chunk, head = head[:maxw], head[maxw:]
                        nop = mybir.InstNoOp(
                            name=f"{inst.name}-ws{k}",
                            sync_info=mybir.SyncInfo(
                                on_wait=list(chunk), on_update=[]
                            ),
                            engine=inst.engine,
                            bass_nofuse=True,
                        )
                        nc.register_instruction(nop)
                        newlist.append(nop)
                        k += 1
                        n += 1
                    si.on_wait.clear()
                    si.on_wait.extend(rest)
                newlist.append(inst)
            insts[:] = newlist
    return n


def _skew_split(tl, dr):
    """Yield the (sbuf_ap, dram_ap) pairs of the 3-DMA engine-skew split of
    one tile transfer (tl: [128, TILE_W] tile, dr: [128, TILE_F] dram view).

    Bb places its two half-chunks at SBUF columns CA+{0, CB/2+PADC} -- the
    gap keeps the AP 3-dim (16 descriptors -> all 16 engines) instead of
    merging back to 8 descriptors (-> engines 0-7 only)."""
    yield tl[:, 0:CA], dr[:, 0:CA]
    yield tl[0:120, CA : CA + CB], dr[0:120, CA:TILE_F]
    h = CB // 2
    sb = tl[120:128, CA : CA + h]
    sb = bass.AP(
        sb.tensor, sb.offset, [list(sb.ap[0]), [h + PADC, 2], [1, h]]
    )
    yield sb, dr[120:128, CA:TILE_F]


def _build_program(shard, tail):
    """One SPMD program: per-core shard [shard] -> out [shard], with the
    global tail [tail] + constants replicated to every core."""
    tailc = tail // 128
    cols = shard // 128
    nf = math.ceil(cols / TILE_F)
    assert cols % TILE_F == 0

    nc = bass.Bass()
    xs = nc.dram_tensor("xs", [shard], F32, kind="ExternalInput")
    xt = nc.dram_tensor("xt", [tail], F32, kind="ExternalInput")
    lts = nc.dram_tensor("lts", [128, 256], F32, kind="ExternalInput")
    ys = nc.dram_tensor("ys", [shard], F32, kind="ExternalOutput")

    xst = xs.rearrange("(p c) -> p c", p=128)
    yst = ys.rearrange("(p c) -> p c", p=128)
    xtt = xt.rearrange("(p c) -> p c", p=128)

    sc_clamp = TARGET_LOG_STD * 1e7
    exp_bias = float(np.log(TARGET_LOG_STD * LN2_T))
    ln2sq = LN2_T * LN2_T

    with tile.TileContext(nc) as tc:
        with (
            tc.tile_pool(name="const", bufs=1) as cpool,
            tc.tile_pool(name="stat", bufs=1) as spool,
            tc.tile_pool(name="psum", bufs=1, space="PSUM") as ppool,
            tc.tile_pool(name="big", bufs=nf) as bpool,
        ):
            # ---------------- phase A: stats over last BUF valid in tail
            # phase-A inputs ride ACT's HWDGE ring (idle at startup) so the
            # first big phase-B load is the very first trigger on SP's ring
            ltst = cpool.tile([128, 256], F32)
            tailt = cpool.tile([128, tailc], F32)
            nc.scalar.dma_start(ltst[:], lts[:])
            nc.scalar.dma_start(tailt[:], xtt[:])

            zer = cpool.tile([128, tailc], F32)
            nc.vector.memset(zer[:], 0.0)

            mask = cpool.tile([128, tailc], F32)
            nc.vector.tensor_scalar(mask[:], tailt[:], 0.0, None, OP.is_gt)
            t1 = cpool.tile([128, tailc], F32)
            nc.vector.tensor_scalar(t1[:], tailt[:], 1.0, None, OP.max)
            lnp = cpool.tile([128, tailc], F32)
            nc.scalar.activation(lnp[:], t1[:], AF.Ln)

            # per-partition inclusive prefix count of valid
            pre = cpool.tile([128, tailc], F32)
            nc.vector.tensor_tensor_scan(
                pre[:], mask[:], zer[:], 0.0, OP.add, OP.add
            )

            # cross-partition exclusive prefix + total, via PE matmuls
            ps_rexc = ppool.tile([128, 1], F32)
            ps_vb = ppool.tile([128, 1], F32)
            last = pre[:, tailc - 1 : tailc]
            nc.tensor.matmul(ps_rexc[:], ltst[:, 0:128], last)
            nc.tensor.matmul(ps_vb[:], ltst[:, 128:256], last)
            rexc = spool.tile([128, 1], F32)
            vb = spool.tile([128, 1], F32)
            nc.vector.tensor_copy(rexc[:], ps_rexc[:])
            nc.vector.tensor_copy(vb[:], ps_vb[:])

            # w = V - rexc - BUF ; select valid lanes with global prefix > w
            w = spool.tile([128, 1], F32)
            nc.vector.tensor_scalar(
                w[:], vb[:], rexc[:, 0:1], float(BUF), OP.subtract, OP.subtract
            )
            selg = cpool.tile([128, tailc], F32)
            nc.vector.tensor_scalar(selg[:], pre[:], w[:, 0:1], None, OP.is_gt)
            sel = cpool.tile([128, tailc], F32)
            nc.vector.tensor_tensor(sel[:], selg[:], mask[:], OP.mult)

            # one-pass moments: cnt, sum(t), sum(t^2) over selected lanes,
            # rows packed into one [128,3] tile -> single broadcast matmul
            stats = spool.tile([128, 3], F32)
            slog = cpool.tile([128, tailc], F32)
            slog2 = cpool.tile([128, tailc], F32)
            nc.vector.tensor_reduce(
                stats[:, 0:1], sel[:], mybir.AxisListType.X, OP.add
            )
            nc.vector.tensor_tensor(slog[:], sel[:], lnp[:], OP.mult)
            nc.vector.tensor_reduce(
                stats[:, 1:2], slog[:], mybir.AxisListType.X, OP.add
            )
            nc.vector.tensor_tensor(slog2[:], slog[:], lnp[:], OP.mult)
            nc.vector.tensor_reduce(
                stats[:, 2:3], slog2[:], mybir.AxisListType.X, OP.add
            )
            ps_st = ppool.tile([128, 3], F32)
            nc.tensor.matmul(ps_st[:], ltst[:, 128:256], stats[:, 0:3])
            bst = spool.tile([128, 3], F32)
            nc.vector.tensor_copy(bst[:], ps_st[:])
            cntb = bst[:, 0:1]
            s1b = bst[:, 1:2]
            s2b = bst[:, 2:3]

            # 1/x via exp(-ln x) on ACT: this walrus rejects the custom-DVE
            # reciprocal encoding, and x (a count >= 1) is exact enough here
            cfl = spool.tile([128, 1], F32)
            nc.vector.tensor_scalar(cfl[:], cntb, 1.0, None, OP.max)
            lncf = spool.tile([128, 1], F32)
            nc.scalar.activation(lncf[:], cfl[:], AF.Ln)
            rcp1 = spool.tile([128, 1], F32)
            nc.scalar.activation(rcp1[:], lncf[:], AF.Exp, scale=-1.0)
            meanl = spool.tile([128, 1], F32)
            nc.vector.tensor_tensor(meanl[:], s1b, rcp1[:], OP.mult)

            # unbiased variance, one-pass: (s2 - s1*mean) / max(cnt-1, 1)
            smean = spool.tile([128, 1], F32)
            nc.vector.tensor_tensor(smean[:], s1b, meanl[:], OP.mult)
            diff = spool.tile([128, 1], F32)
            nc.vector.tensor_tensor(diff[:], s2b, smean[:], OP.subtract)
            diffc = spool.tile([128, 1], F32)
            nc.vector.tensor_scalar(diffc[:], diff[:], 0.0, None, OP.max)

            den = spool.tile([128, 1], F32)
            nc.vector.tensor_scalar(
                den[:], cntb, 1.0, 1.0, OP.subtract, OP.max
            )
            lnden = spool.tile([128, 1], F32)
            nc.scalar.activation(lnden[:], den[:], AF.Ln)
            rcp2 = spool.tile([128, 1], F32)
            nc.scalar.activation(rcp2[:], lnden[:], AF.Exp, scale=-1.0)
            varl = spool.tile([128, 1], F32)
            nc.vector.tensor_tensor(varl[:], diffc[:], rcp2[:], OP.mult)

            # count<=1 -> std2 := 1  (stdL := ln2), via varL += ind*ln2^2
            ind = spool.tile([128, 1], F32)
            nc.vector.tensor_scalar(
                ind[:], cntb, 1.5, ln2sq, OP.is_lt, OP.mult
            )
            varp = spool.tile([128, 1], F32)
            nc.vector.tensor_tensor(varp[:], varl[:], ind[:], OP.add)

            # sc = TLS*ln2/sqrt(varp) = exp(-0.5*ln(varp) + ln(TLS*ln2))
            lnv = spool.tile([128, 1], F32)
            nc.scalar.activation(lnv[:], varp[:], AF.Ln)
            ebias = spool.tile([128, 1], F32)
            nc.vector.memset(ebias[:], exp_bias)
            sc_r = spool.tile([128, 1], F32)
            nc.scalar.activation(
                sc_r[:], lnv[:], AF.Exp, scale=-0.5, bias=ebias[:, 0:1]
            )
            sc = spool.tile([128, 1], F32)
            nc.vector.tensor_scalar(sc[:], sc_r[:], sc_clamp, None, OP.min)
            mb = spool.tile([128, 1], F32)
            nc.vector.tensor_tensor(mb[:], meanl[:], sc[:], OP.mult)
            bi = spool.tile([128, 1], F32)
            nc.vector.tensor_scalar(
                bi[:], mb[:], -1.0, LN2_T * TARGET_LOG_MEAN, OP.mult, OP.add
            )

            # ---------------- phase B: streamed elementwise map
            tiles = []
            for i in range(nf):
                f0 = i * TILE_F
                tl = bpool.tile([128, TILE_W], F32, tag="work")
                for sb, dr in _skew_split(tl, xst[:, f0 : f0 + TILE_F]):
                    nc.sync.dma_start(sb, dr)
                tiles.append(tl)
            for tl in tiles:
                nc.scalar.activation(tl[:, :], tl[:, :], AF.Ln)
                nc.scalar.activation(
                    tl[:, :], tl[:, :], AF.Exp, scale=sc[:, 0:1], bias=bi[:, 0:1]
                )
            for i, tl in enumerate(tiles):
                f0 = i * TILE_F
                for sb, dr in _skew_split(tl, yst[:, f0 : f0 + TILE_F]):
                    nc.sync.dma_start(dr, sb)

    _legalize_sync_waits(nc)
    nc.finalize()
    return nc


_cache = {}


def _get_program(shard, tail):
    key = (shard, tail)
    if key not in _cache:
        _cache[key] = _build_program(shard, tail)
    return _cache[key]


def _consts():
    k = np.arange(128, dtype=np.float32)
    lt_strict = (k[:, None] < k[None, :]).astype(np.float32)  # [k, m]: k < m
    ones = np.ones((128, 128), np.float32)
    return np.concatenate([lt_strict, ones], axis=1)


def _prep(x):
    """Build (nc, in_maps) for the full input x."""
    n = x.shape[0]
    shard = n // N_CORES
    assert n % (N_CORES * 128) == 0, f"unsupported size {n}"

    # tail window guaranteed to contain the last BUF valid elements
    tail = 16384
    while tail < n and int(np.count_nonzero(x[n - tail :] > 0.0)) < BUF:
        tail *= 2
    tail = min(tail, n)
    # phase-A SBUF tiles scale with the tail; beyond 2^16 elements they
    # would not fit alongside the streaming pool
    if tail > (1 << 16):
        # pathological density: synthesize an equivalent tail on the host
        # holding the last <=BUF valid values (stats are order-independent)
        vals = x[x > 0.0]
        kv = vals[-BUF:] if vals.size > BUF else vals
        tail = 16384
        fake = np.zeros(tail, np.float32)
        if kv.size:
            fake[-kv.size :] = kv
        xt = fake
    else:
        xt = x[n - tail :]

    nc = _get_program(shard, tail)
    consts = _consts()
    in_maps = [
        {
            "xs": x[c * shard : (c + 1) * shard],
            "xt": xt,
            "lts": consts,
        }
        for c in range(N_CORES)
    ]
    return nc, in_maps


def _assemble(res):
    return np.concatenate(
        [res.results[c]["ys"] for c in range(N_CORES)]
    )


def kernel(pitch_values):
    x = np.ascontiguousarray(np.asarray(pitch_values, dtype=np.float32))
    nc, in_maps = _prep(x)
    res = run_bass_kernel_spmd(nc, in_maps, core_ids=list(range(N_CORES)))
    return _assemble(res)
